# revision 36
# baseline (speedup 1.0000x reference)
"""CapsuleLayer forward on 8 Trainium2 NeuronCores.

The reference collapses algebraically: routing runs exactly one iteration
with uniform coefficients c = 1/R, so

    s[b, (n,o)] = (1/R) * sum_{r,i} x[b,r,i] * W[n,r,i,o]
                = (1/R) * (x_flat @ W_flat)[b, (n,o)]
    v = squash(s) over o

i.e. one [256, 9216] @ [9216, 160] matmul plus a tiny squash on 40960
elements. u_hat ([10,256,1152,16], 189 MB) is never materialized.

Sharding: contraction-dim (K = R*CIN) sharding across the 8 cores — each
core reads only 1/8 of x^T and 1/8 of W (1.9 MB vs 7 MB/core for batch
sharding) and the PE does 4x fewer matmul instructions (full 128-wide
stationary operand). Each core emits its raw partial product (s^T layout,
[160, 256]); the host sums the 8 partials and applies the tiny squash
while unsharding. (An on-device ReduceScatter was measured at ~46 us —
a 32 us launch-skew barrier plus 14 us transfer — so cross-core
reduction on device is strictly worse.)

Matmuls run as float32r (fp32 bits, replicated PE mode): with a 256-wide
moving operand this runs at 1 cycle/row vs 4 for plain fp32.
"""

import numpy as np
from contextlib import ExitStack

import concourse.tile as tile
from concourse import bacc, mybir
from concourse.bass_utils import run_bass_kernel_spmd

N_CAPS, R, CIN, COUT = 10, 1152, 8, 16
B = 256
NCORES = 8
K = R * CIN            # 9216 contraction length
KSH = K // NCORES      # 1152 contraction slice per core
NO = N_CAPS * COUT     # 160 output rows (s^T layout)
P = 128
KT = KSH // P          # 9 k-tiles per core

F32 = mybir.dt.float32
F32R = mybir.dt.float32r

_built = None


# Pipelined input chunks: chunk 0 covers k-tiles [0, KSPLIT), chunk 1 the
# rest. x^T and W are host-packed into ONE combined tensor whose rows are
# chunk-contiguous, so each chunk loads as two large-descriptor DMAs on
# the two 64-aligned partition bands (sync + gpsimd queues). Per-queue
# throughput is descriptor-size-bound (~400 ns + bytes/23 GB/s per
# descriptor per SDMA engine), so ~8 KB descriptors on 2 queues saturate
# HBM where 3 queues of 2-6 KB descriptors could not. Partition bands
# NOT aligned to 64 degenerate into 4-byte descriptors (~15 GB/s —
# measured), so only 0:64/64:128 are used.
KSPLIT = 5
C0 = KSPLIT * (B + NO)            # chunk 0 column count (2080)
C1 = (KT - KSPLIT) * (B + NO)     # chunk 1 column count (1664)


def _col_xt(j):
    if j < KSPLIT:
        return j * B
    return C0 + (j - KSPLIT) * B


def _col_wk(j):
    if j < KSPLIT:
        return KSPLIT * B + j * NO
    return C0 + (KT - KSPLIT) * B + (j - KSPLIT) * NO


def _build_nc():
    nc = bacc.Bacc(
        "TRN2", target_bir_lowering=False, debug=False, num_devices=NCORES
    )
    comb = nc.dram_tensor("comb", [P, C0 + C1], F32R, kind="ExternalInput").ap()
    out = nc.dram_tensor("out", [NO, B], F32, kind="ExternalOutput").ap()

    with tile.TileContext(nc) as tc, ExitStack() as ctx:
        xp = ctx.enter_context(tc.tile_pool(name="xp", bufs=2))
        wp = ctx.enter_context(tc.tile_pool(name="wp", bufs=2))
        pp = ctx.enter_context(tc.tile_pool(name="pp", bufs=1, space="PSUM"))

        # PE warm-up: the HAM clock gate keeps the PE at 1.2 GHz until
        # it has seen ~3.4 us of sustained activity. Spend the DMA-fill
        # window running throwaway matmuls on scratch tiles so the real
        # matmuls run at 2.4 GHz.
        warm = ctx.enter_context(tc.tile_pool(name="warm", bufs=1))
        wsrc = warm.tile([P, P], F32, tag="wsrc")
        wps = pp.tile([P, P], F32, tag="wps")
        nc.gpsimd.memset(wsrc[:], 0.0)
        for _ in range(12):
            nc.tensor.matmul(wps[:], wsrc[:], wsrc[:], start=True, stop=True)

        # s^T partial: [160, 256] across two PSUM tiles (stationary W
        # tile is limited to 128 columns).
        psA = pp.tile([P, B], F32, tag="psA")
        psB = pp.tile([NO - P, B], F32, tag="psB")

        # Two pipelined chunks, each loaded by two chunk-contiguous
        # band DMAs (~8 KB descriptors).
        comb_t = xp.tile([P, C0 + C1], F32R)
        H = P // 2
        for c_lo, c_hi in ((0, C0), (C0, C0 + C1)):
            nc.sync.dma_start(
                comb_t[0:H, c_lo:c_hi], comb[0:H, c_lo:c_hi]
            )
            nc.gpsimd.dma_start(
                comb_t[H:P, c_lo:c_hi], comb[H:P, c_lo:c_hi]
            )

        # Matmuls trail chunk arrival; within each chunk psA's group runs
        # first so its PSUM eviction overlaps psB's remaining matmuls.
        order = (
            [(j, 0) for j in range(KSPLIT)]
            + [(j, 1) for j in range(KSPLIT)]
            + [(j, 0) for j in range(KSPLIT, KT)]
            + [(j, 1) for j in range(KSPLIT, KT)]
        )
        for j, h in order:
            cx, cw = _col_xt(j), _col_wk(j)
            if h == 0:
                nc.tensor.matmul(
                    psA[:],
                    comb_t[:, cw:cw + P],
                    comb_t[:, cx:cx + B],
                    start=(j == 0), stop=(j == KT - 1),
                )
            else:
                nc.tensor.matmul(
                    psB[:],
                    comb_t[:, cw + P:cw + NO],
                    comb_t[:, cx:cx + B],
                    start=(j == 0), stop=(j == KT - 1),
                )
        sb = ctx.enter_context(tc.tile_pool(name="sb", bufs=1))
        sA = sb.tile([P, B], F32, tag="sA")
        sB = sb.tile([NO - P, B], F32, tag="sB")
        nc.vector.tensor_copy(sA[:], psA[:])
        nc.vector.tensor_copy(sB[:], psB[:])
        nc.sync.dma_start(out[0:H, :], sA[0:H, :])
        nc.gpsimd.dma_start(out[H:P, :], sA[H:P, :])
        nc.scalar.dma_start(out[P:NO, :], sB[:])

    nc.compile()
    return nc


def _get_nc():
    global _built
    if _built is None:
        _built = _build_nc()
    return _built


def _make_in_maps(x, W):
    x = np.asarray(x, dtype=np.float32)
    W = np.asarray(W, dtype=np.float32)
    # x^T: [K, B]; W to [K, NO] with k = r*CIN + i matching x's flattening.
    # Then pack k-tile-major per core: [NCORES, P, KT * cols] where row p
    # holds k-tile k's p-th contraction row at column block k.
    xt_full = x.reshape(B, K).T  # [K, B] view
    wk_full = W.transpose(1, 2, 0, 3).reshape(K, NO)
    xt_pack = xt_full.reshape(NCORES, KT, P, B).transpose(0, 2, 1, 3)
    wk_pack = wk_full.reshape(NCORES, KT, P, NO).transpose(0, 2, 1, 3)
    comb = np.ascontiguousarray(
        np.concatenate(
            [
                xt_pack[:, :, :KSPLIT].reshape(NCORES, P, -1),
                wk_pack[:, :, :KSPLIT].reshape(NCORES, P, -1),
                xt_pack[:, :, KSPLIT:].reshape(NCORES, P, -1),
                wk_pack[:, :, KSPLIT:].reshape(NCORES, P, -1),
            ],
            axis=2,
        )
    )
    return [{"comb": comb[c]} for c in range(NCORES)]


def _assemble(results):
    # Sum the 8 K-slice partials (the "all-reduce" leg of unsharding),
    # then apply squash: with t = raw sum (s = t/R, ssq = sum_o t^2),
    #   v = t * sqrt(ssq) / (R^2 + ssq)
    t = np.zeros((NO, B), dtype=np.float32)
    for c in range(NCORES):
        t += results[c]["out"]
    t = t.T.reshape(B, N_CAPS, COUT).astype(np.float64)
    ssq = np.sum(t * t, axis=-1, keepdims=True)
    v = t * np.sqrt(ssq) / (R * R + ssq)
    return np.ascontiguousarray(
        v.transpose(1, 0, 2)[:, :, None, None, :]
    ).astype(np.float32)


def _run(x, W, **spmd_kwargs):
    nc = _get_nc()
    in_maps = _make_in_maps(x, W)
    return run_bass_kernel_spmd(nc, in_maps, list(range(NCORES)), **spmd_kwargs)


def kernel(x, W):
    res = _run(x, W)
    return _assemble(res.results)


# revision 39
# speedup vs baseline: 1.0283x; 1.0283x over previous
"""CapsuleLayer forward on 8 Trainium2 NeuronCores.

The reference collapses algebraically: routing runs exactly one iteration
with uniform coefficients c = 1/R, so

    s[b, (n,o)] = (1/R) * sum_{r,i} x[b,r,i] * W[n,r,i,o]
                = (1/R) * (x_flat @ W_flat)[b, (n,o)]
    v = squash(s) over o

i.e. one [256, 9216] @ [9216, 160] matmul plus a tiny squash on 40960
elements. u_hat ([10,256,1152,16], 189 MB) is never materialized.

Sharding: contraction-dim (K = R*CIN) sharding across the 8 cores — each
core reads only 1/8 of x^T and 1/8 of W (1.9 MB vs 7 MB/core for batch
sharding) and the PE does 4x fewer matmul instructions (full 128-wide
stationary operand). Each core emits its raw partial product (s^T layout,
[160, 256]); the host sums the 8 partials and applies the tiny squash
while unsharding. (An on-device ReduceScatter was measured at ~46 us —
a 32 us launch-skew barrier plus 14 us transfer — so cross-core
reduction on device is strictly worse.)

Matmuls run as float32r (fp32 bits, replicated PE mode): with a 256-wide
moving operand this runs at 1 cycle/row vs 4 for plain fp32.
"""

import numpy as np
from contextlib import ExitStack

import concourse.tile as tile
from concourse import bacc, mybir
from concourse.bass_utils import run_bass_kernel_spmd

N_CAPS, R, CIN, COUT = 10, 1152, 8, 16
B = 256
NCORES = 8
K = R * CIN            # 9216 contraction length
KSH = K // NCORES      # 1152 contraction slice per core
NO = N_CAPS * COUT     # 160 output rows (s^T layout)
P = 128
KT = KSH // P          # 9 k-tiles per core

F32 = mybir.dt.float32
F32R = mybir.dt.float32r

_built = None


# Pipelined input chunks: chunk 0 covers k-tiles [0, KSPLIT), chunk 1 the
# rest. x^T is split into 64-aligned partition bands (one per HW queue);
# partition bands NOT aligned to 64 degenerate into 4-byte DMA
# descriptors (~15 GB/s — measured), so only 0:64/64:128 are used.
KSPLIT = 5


def _build_nc():
    nc = bacc.Bacc(
        "TRN2", target_bir_lowering=False, debug=False, num_devices=NCORES
    )
    xt = nc.dram_tensor("xt", [P, KT * B], F32R, kind="ExternalInput").ap()
    wk = nc.dram_tensor("wk", [P, KT * NO], F32R, kind="ExternalInput").ap()
    out = nc.dram_tensor("out", [NO, B], F32, kind="ExternalOutput").ap()

    with tile.TileContext(nc) as tc, ExitStack() as ctx:
        xp = ctx.enter_context(tc.tile_pool(name="xp", bufs=2))
        wp = ctx.enter_context(tc.tile_pool(name="wp", bufs=2))
        pp = ctx.enter_context(tc.tile_pool(name="pp", bufs=1, space="PSUM"))

        # PE warm-up: the HAM clock gate keeps the PE at 1.2 GHz until
        # it has seen ~3.4 us of sustained activity. Spend the DMA-fill
        # window running throwaway matmuls on scratch tiles so the real
        # matmuls run at 2.4 GHz.
        warm = ctx.enter_context(tc.tile_pool(name="warm", bufs=1))
        wsrc = warm.tile([P, P], F32, tag="wsrc")
        wps = pp.tile([P, P], F32, tag="wps")
        nc.gpsimd.memset(wsrc[:], 0.0)
        for _ in range(12):
            nc.tensor.matmul(wps[:], wsrc[:], wsrc[:], start=True, stop=True)

        # s^T partial: [160, 256] across two PSUM tiles (stationary W
        # tile is limited to 128 columns).
        psA = pp.tile([P, B], F32, tag="psA")
        psB = pp.tile([NO - P, B], F32, tag="psB")

        # Two pipelined chunks; within each chunk x^T rides two HW queues
        # (64-aligned partition bands) and W the third. Large contiguous
        # per-partition descriptors keep per-queue throughput up.
        xt_t = xp.tile([P, KT * B], F32R)
        wk_t = wp.tile([P, KT * NO], F32R)
        H = P // 2
        for k0, k1 in ((0, KSPLIT), (KSPLIT, KT)):
            nc.sync.dma_start(
                xt_t[0:H, k0 * B:k1 * B], xt[0:H, k0 * B:k1 * B]
            )
            nc.gpsimd.dma_start(
                xt_t[H:P, k0 * B:k1 * B], xt[H:P, k0 * B:k1 * B]
            )
            nc.scalar.dma_start(
                wk_t[:, k0 * NO:k1 * NO], wk[:, k0 * NO:k1 * NO]
            )

        # Matmuls trail chunk arrival; within each chunk psA's group runs
        # first so its PSUM eviction overlaps psB's remaining matmuls.
        order = (
            [(j, 0) for j in range(KSPLIT)]
            + [(j, 1) for j in range(KSPLIT)]
            + [(j, 0) for j in range(KSPLIT, KT)]
            + [(j, 1) for j in range(KSPLIT, KT)]
        )
        for j, h in order:
            if h == 0:
                nc.tensor.matmul(
                    psA[:],
                    wk_t[:, j * NO:j * NO + P],
                    xt_t[:, j * B:(j + 1) * B],
                    start=(j == 0), stop=(j == KT - 1),
                )
            else:
                nc.tensor.matmul(
                    psB[:],
                    wk_t[:, j * NO + P:(j + 1) * NO],
                    xt_t[:, j * B:(j + 1) * B],
                    start=(j == 0), stop=(j == KT - 1),
                )
        sb = ctx.enter_context(tc.tile_pool(name="sb", bufs=1))
        sA = sb.tile([P, B], F32, tag="sA")
        sB = sb.tile([NO - P, B], F32, tag="sB")
        nc.vector.tensor_copy(sA[:], psA[:])
        nc.vector.tensor_copy(sB[:], psB[:])
        nc.sync.dma_start(out[0:H, :], sA[0:H, :])
        nc.gpsimd.dma_start(out[H:P, :], sA[H:P, :])
        nc.scalar.dma_start(out[P:NO, :], sB[:])

    nc.compile()
    return nc


def _get_nc():
    global _built
    if _built is None:
        _built = _build_nc()
    return _built


def _make_in_maps(x, W):
    x = np.asarray(x, dtype=np.float32)
    W = np.asarray(W, dtype=np.float32)
    # x^T: [K, B]; W to [K, NO] with k = r*CIN + i matching x's flattening.
    # Then pack k-tile-major per core: [NCORES, P, KT * cols] where row p
    # holds k-tile k's p-th contraction row at column block k.
    xt_full = x.reshape(B, K).T  # [K, B] view
    wk_full = W.transpose(1, 2, 0, 3).reshape(K, NO)
    xt_pack = np.ascontiguousarray(
        xt_full.reshape(NCORES, KT, P, B).transpose(0, 2, 1, 3)
    ).reshape(NCORES, P, KT * B)
    wk_pack = np.ascontiguousarray(
        wk_full.reshape(NCORES, KT, P, NO).transpose(0, 2, 1, 3)
    ).reshape(NCORES, P, KT * NO)
    return [{"xt": xt_pack[c], "wk": wk_pack[c]} for c in range(NCORES)]


def _assemble(results):
    # Sum the 8 K-slice partials (the "all-reduce" leg of unsharding),
    # then apply squash: with t = raw sum (s = t/R, ssq = sum_o t^2),
    #   v = t * sqrt(ssq) / (R^2 + ssq)
    t = np.zeros((NO, B), dtype=np.float32)
    for c in range(NCORES):
        t += results[c]["out"]
    t = t.T.reshape(B, N_CAPS, COUT).astype(np.float64)
    ssq = np.sum(t * t, axis=-1, keepdims=True)
    v = t * np.sqrt(ssq) / (R * R + ssq)
    return np.ascontiguousarray(
        v.transpose(1, 0, 2)[:, :, None, None, :]
    ).astype(np.float32)


def _run(x, W, **spmd_kwargs):
    nc = _get_nc()
    in_maps = _make_in_maps(x, W)
    return run_bass_kernel_spmd(nc, in_maps, list(range(NCORES)), **spmd_kwargs)


def kernel(x, W):
    global _built
    try:
        res = _run(x, W)
    except Exception:
        # Transient device hiccup (rare first-execution
        # NRT_EXEC_UNIT_UNRECOVERABLE resets the core) — retry once.
        import time

        time.sleep(2.0)
        _built = None
        res = _run(x, W)
    return _assemble(res.results)


# revision 41
# speedup vs baseline: 1.1189x; 1.0881x over previous
"""CapsuleLayer forward on 8 Trainium2 NeuronCores.

The reference collapses algebraically: routing runs exactly one iteration
with uniform coefficients c = 1/R, so

    s[b, (n,o)] = (1/R) * sum_{r,i} x[b,r,i] * W[n,r,i,o]
                = (1/R) * (x_flat @ W_flat)[b, (n,o)]
    v = squash(s) over o

i.e. one [256, 9216] @ [9216, 160] matmul plus a tiny squash on 40960
elements. u_hat ([10,256,1152,16], 189 MB) is never materialized.

Sharding: contraction-dim (K = R*CIN) sharding across the 8 cores — each
core reads only 1/8 of x^T and 1/8 of W (1.9 MB vs 7 MB/core for batch
sharding) and the PE does 4x fewer matmul instructions (full 128-wide
stationary operand). Each core emits its raw partial product (s^T layout,
[160, 256]); the host sums the 8 partials and applies the tiny squash
while unsharding. (An on-device ReduceScatter was measured at ~46 us —
a 32 us launch-skew barrier plus 14 us transfer — so cross-core
reduction on device is strictly worse.)

Matmuls run as float32r (fp32 bits, replicated PE mode): with a 256-wide
moving operand this runs at 1 cycle/row vs 4 for plain fp32.
"""

import numpy as np
from contextlib import ExitStack

import concourse.tile as tile
from concourse import bacc, mybir
from concourse.bass_utils import run_bass_kernel_spmd

N_CAPS, R, CIN, COUT = 10, 1152, 8, 16
B = 256
NCORES = 8
K = R * CIN            # 9216 contraction length
KSH = K // NCORES      # 1152 contraction slice per core
NO = N_CAPS * COUT     # 160 output rows (s^T layout)
P = 128
KT = KSH // P          # 9 k-tiles per core

F32 = mybir.dt.float32
F32R = mybir.dt.float32r

_built = None


# Pipelined input chunks: chunk 0 covers k-tiles [0, KSPLIT), chunk 1 the
# rest. x^T is split into 64-aligned partition bands (one per HW queue);
# partition bands NOT aligned to 64 degenerate into 4-byte DMA
# descriptors (~15 GB/s — measured), so only 0:64/64:128 are used.
KSPLIT = 5


def _build_nc():
    nc = bacc.Bacc(
        "TRN2", target_bir_lowering=False, debug=False, num_devices=NCORES
    )
    xt = nc.dram_tensor("xt", [P, KT * B], F32R, kind="ExternalInput").ap()
    wk = nc.dram_tensor("wk", [P, KT * NO], F32R, kind="ExternalInput").ap()
    out = nc.dram_tensor("out", [NO, B], F32, kind="ExternalOutput").ap()

    with tile.TileContext(nc) as tc, ExitStack() as ctx:
        xp = ctx.enter_context(tc.tile_pool(name="xp", bufs=2))
        wp = ctx.enter_context(tc.tile_pool(name="wp", bufs=2))
        pp = ctx.enter_context(tc.tile_pool(name="pp", bufs=1, space="PSUM"))

        # PE warm-up: the HAM clock gate keeps the PE at 1.2 GHz until
        # it has seen ~3.4 us of sustained activity. Spend the DMA-fill
        # window running throwaway matmuls on scratch tiles so the real
        # matmuls run at 2.4 GHz.
        warm = ctx.enter_context(tc.tile_pool(name="warm", bufs=1))
        wsrc = warm.tile([P, P], F32, tag="wsrc")
        wps = pp.tile([P, P], F32, tag="wps")
        nc.gpsimd.memset(wsrc[:], 0.0)
        for _ in range(12):
            nc.tensor.matmul(wps[:], wsrc[:], wsrc[:], start=True, stop=True)

        # s^T partial: [160, 256] across two PSUM tiles (stationary W
        # tile is limited to 128 columns).
        psA = pp.tile([P, B], F32, tag="psA")
        psB = pp.tile([NO - P, B], F32, tag="psB")

        # Two pipelined chunks; within each chunk x^T rides two HW queues
        # (64-aligned partition bands) and W the third. Large contiguous
        # per-partition descriptors keep per-queue throughput up.
        xt_t = xp.tile([P, KT * B], F32R)
        wk_t = wp.tile([P, KT * NO], F32R)
        H = P // 2
        for k0, k1 in ((0, KSPLIT), (KSPLIT, KT)):
            nc.sync.dma_start(
                xt_t[0:H, k0 * B:k1 * B], xt[0:H, k0 * B:k1 * B]
            )
            nc.gpsimd.dma_start(
                xt_t[H:P, k0 * B:k1 * B], xt[H:P, k0 * B:k1 * B]
            )
            nc.scalar.dma_start(
                wk_t[:, k0 * NO:k1 * NO], wk[:, k0 * NO:k1 * NO]
            )

        # Matmuls trail chunk arrival; within each chunk psA's group runs
        # first so its PSUM eviction overlaps psB's remaining matmuls.
        order = (
            [(j, 0) for j in range(KSPLIT)]
            + [(j, 1) for j in range(KSPLIT)]
            + [(j, 0) for j in range(KSPLIT, KT)]
            + [(j, 1) for j in range(KSPLIT, KT)]
        )
        for j, h in order:
            if h == 0:
                nc.tensor.matmul(
                    psA[:],
                    wk_t[:, j * NO:j * NO + P],
                    xt_t[:, j * B:(j + 1) * B],
                    start=(j == 0), stop=(j == KT - 1),
                )
            else:
                nc.tensor.matmul(
                    psB[:],
                    wk_t[:, j * NO + P:(j + 1) * NO],
                    xt_t[:, j * B:(j + 1) * B],
                    start=(j == 0), stop=(j == KT - 1),
                )
        sb = ctx.enter_context(tc.tile_pool(name="sb", bufs=1))
        sA = sb.tile([P, B], F32, tag="sA")
        sB = sb.tile([NO - P, B], F32, tag="sB")
        nc.vector.tensor_copy(sA[:], psA[:])
        nc.vector.tensor_copy(sB[:], psB[:])
        nc.sync.dma_start(out[0:H, :], sA[0:H, :])
        nc.gpsimd.dma_start(out[H:P, :], sA[H:P, :])
        nc.scalar.dma_start(out[P:NO, :], sB[:])

    nc.compile()
    return nc


def _get_nc():
    global _built
    if _built is None:
        _built = _build_nc()
    return _built


def _make_in_maps(x, W):
    x = np.asarray(x, dtype=np.float32)
    W = np.asarray(W, dtype=np.float32)
    # x^T: [K, B]; W to [K, NO] with k = r*CIN + i matching x's flattening.
    # Then pack k-tile-major per core: [NCORES, P, KT * cols] where row p
    # holds k-tile k's p-th contraction row at column block k.
    xt_full = x.reshape(B, K).T  # [K, B] view
    wk_full = W.transpose(1, 2, 0, 3).reshape(K, NO)
    xt_pack = np.ascontiguousarray(
        xt_full.reshape(NCORES, KT, P, B).transpose(0, 2, 1, 3)
    ).reshape(NCORES, P, KT * B)
    wk_pack = np.ascontiguousarray(
        wk_full.reshape(NCORES, KT, P, NO).transpose(0, 2, 1, 3)
    ).reshape(NCORES, P, KT * NO)
    return [{"xt": xt_pack[c], "wk": wk_pack[c]} for c in range(NCORES)]


def _assemble(results):
    # Sum the 8 K-slice partials (the "all-reduce" leg of unsharding),
    # then apply squash: with t = raw sum (s = t/R, ssq = sum_o t^2),
    #   v = t * sqrt(ssq) / (R^2 + ssq)
    t = np.zeros((NO, B), dtype=np.float32)
    for c in range(NCORES):
        t += results[c]["out"]
    t = t.T.reshape(B, N_CAPS, COUT).astype(np.float64)
    ssq = np.sum(t * t, axis=-1, keepdims=True)
    v = t * np.sqrt(ssq) / (R * R + ssq)
    return np.ascontiguousarray(
        v.transpose(1, 0, 2)[:, :, None, None, :]
    ).astype(np.float32)


def _run(x, W, **spmd_kwargs):
    nc = _get_nc()
    in_maps = _make_in_maps(x, W)
    return run_bass_kernel_spmd(nc, in_maps, list(range(NCORES)), **spmd_kwargs)


def kernel(x, W):
    global _built
    try:
        res = _run(x, W)
    except Exception:
        # Transient device hiccup (rare first-execution
        # NRT_EXEC_UNIT_UNRECOVERABLE resets the core) — retry once.
        import time

        time.sleep(2.0)
        _built = None
        res = _run(x, W)
    return _assemble(res.results)
